# revision 2
# baseline (speedup 1.0000x reference)
"""CRF NLL loss kernel for Trainium2 (8 NeuronCores, SPMD data-parallel over batch).

Math: loss = mean_b( logZ_b - gold_b ) for a linear-chain CRF with
H = 52 states (50 tags + START/STOP), T = 512, B = 64, F = 1024.

Strategy per core (8 sequences each):
  - emit = features @ W.T     : PE matmul, K=F contracted in 8 chunks of 128,
    moving operand = host-pretransposed features [F, (t,b)].
  - forward algorithm in the exp domain:  q_{t+1} = exp(emit_t) ⊙ (E' q_t)
    with E' = exp(transition - c) as the PE stationary operand, so each step
    is one tiny matmul + one vector multiply. Renormalize q by its column sum
    every RENORM steps (reciprocal saved; host recovers log Z exactly).
  - raw emit PSUM is DMA'd back; the gold-score gather (pure index math) and
    the final scalar assembly happen on host.
"""

import os
import numpy as np

B, T, F, NT = 64, 512, 1024, 50
H = NT + 2
START, STOP = H - 2, H - 1
NEG = -100000000.0

NCORES = 8
BL = B // NCORES           # 8 sequences per core
TCHUNK = 64                # t-steps per emit tile (TCHUNK * BL = 512 free)
NTILES = T // TCHUNK       # 8 emit tiles per core
KC = F // 128              # 8 contraction chunks
RENORM = 64                # renormalize every RENORM scan steps
NREN = T // RENORM         # 8 renorms

_CACHE = {}


def _build_program():
    import concourse.bacc as bacc
    import concourse.tile as tile
    import concourse.mybir as mybir
    from concourse.bass import ts

    f32 = mybir.dt.float32
    AF = mybir.ActivationFunctionType
    nc = bacc.Bacc("TRN2", target_bir_lowering=False, debug=False)

    feats = nc.dram_tensor("feats", [F, T * BL], f32, kind="ExternalInput")
    wt = nc.dram_tensor("wt", [F, H], f32, kind="ExternalInput")
    est = nc.dram_tensor("est", [H, H], f32, kind="ExternalInput")
    q0d = nc.dram_tensor("q0", [H, BL], f32, kind="ExternalInput")
    bcol = nc.dram_tensor("bcol", [H, 1], f32, kind="ExternalInput")
    ones_h1 = nc.dram_tensor("ones_h1", [H, 1], f32, kind="ExternalInput")
    ones_1h = nc.dram_tensor("ones_1h", [1, H], f32, kind="ExternalInput")

    emit_out = nc.dram_tensor("emit", [H, T * BL], f32, kind="ExternalOutput")
    qfin_out = nc.dram_tensor("qfin", [H, BL], f32, kind="ExternalOutput")
    rhist_out = nc.dram_tensor("rhist", [1, NREN, BL], f32, kind="ExternalOutput")

    feats_r = feats.ap().rearrange(
        "(kc p) (j n) -> j p kc n", kc=KC, n=TCHUNK * BL
    )

    with tile.TileContext(nc) as tc:
        with (
            tc.tile_pool(name="singles", bufs=1) as singles,
            tc.tile_pool(name="fpool", bufs=3) as fpool,
            tc.tile_pool(name="qpool", bufs=4) as qpool,
            tc.tile_pool(name="eps_ps", bufs=2, space="PSUM") as eps_ps,
            tc.tile_pool(name="q_ps", bufs=3, space="PSUM") as q_ps,
            tc.tile_pool(name="z_ps", bufs=1, space="PSUM") as z_ps,
            tc.tile_pool(name="bc_ps", bufs=1, space="PSUM") as bc_ps,
        ):
            wt_sb = singles.tile([128, KC, H], f32)
            nc.sync.dma_start(wt_sb[:], wt.ap().rearrange("(kc p) h -> p kc h", kc=KC))
            est_sb = singles.tile([H, H], f32)
            nc.sync.dma_start(est_sb[:], est.ap())
            q0_sb = singles.tile([H, BL], f32)
            nc.sync.dma_start(q0_sb[:], q0d.ap())
            b_sb = singles.tile([H, 1], f32)
            nc.sync.dma_start(b_sb[:], bcol.ap())
            oh1_sb = singles.tile([H, 1], f32)
            nc.sync.dma_start(oh1_sb[:], ones_h1.ap())
            o1h_sb = singles.tile([1, H], f32)
            nc.sync.dma_start(o1h_sb[:], ones_1h.ap())
            rhist_sb = singles.tile([1, NREN, BL], f32)
            eemit_sb = singles.tile([H, T, BL], f32)

            # ---- emit projection, one tile per TCHUNK of steps ----
            for j in range(NTILES):
                ft = fpool.tile([128, KC, TCHUNK * BL], f32)
                nc.sync.dma_start(ft[:], feats_r[j])
                eps = eps_ps.tile([H, TCHUNK * BL], f32)
                for kc in range(KC):
                    nc.tensor.matmul(
                        eps[:],
                        wt_sb[:, kc, :],
                        ft[:, kc, :],
                        start=(kc == 0),
                        stop=(kc == KC - 1),
                    )
                # eemit[:, j*TCHUNK:(j+1)*TCHUNK, :] = exp(eps + b)
                nc.scalar.activation(
                    eemit_sb[:, ts(j, TCHUNK), :],
                    eps.rearrange("p (t b) -> p t b", b=BL),
                    AF.Exp,
                    bias=b_sb[:],
                )
                # raw emit (pre-bias) back to DRAM for host-side gold gather
                em_sb = fpool.tile([H, TCHUNK * BL], f32, tag="emit_stage")
                nc.scalar.copy(em_sb[:], eps[:])
                nc.sync.dma_start(emit_out.ap()[:, ts(j, TCHUNK * BL)], em_sb[:])

            # ---- forward scan ----
            q = q0_sb
            for t in range(T):
                ps = q_ps.tile([H, BL], f32)
                nc.tensor.matmul(ps[:], est_sb[:], q[:], start=True, stop=True)
                qn = qpool.tile([H, BL], f32)
                nc.vector.tensor_mul(qn[:], ps[:], eemit_sb[:, t, :])
                q = qn
                if (t + 1) % RENORM == 0:
                    k = (t + 1) // RENORM - 1
                    zs = z_ps.tile([1, BL], f32)
                    nc.tensor.matmul(zs[:], oh1_sb[:], q[:], start=True, stop=True)
                    nc.vector.reciprocal(rhist_sb[:, k, :], zs[:])
                    bc = bc_ps.tile([H, BL], f32)
                    nc.tensor.matmul(
                        bc[:], o1h_sb[:], rhist_sb[:, k, :], start=True, stop=True
                    )
                    qr = qpool.tile([H, BL], f32)
                    nc.vector.tensor_mul(qr[:], bc[:], q[:])
                    q = qr

            nc.sync.dma_start(qfin_out.ap(), q[:])
            nc.sync.dma_start(rhist_out.ap(), rhist_sb[:])

    nc.compile()
    return nc


def _get_program():
    if "nc" not in _CACHE:
        _CACHE["nc"] = _build_program()
    return _CACHE["nc"]


def kernel(features, W, b, transition, masks, tags):
    from concourse.bass_utils import run_bass_kernel_spmd

    features = np.asarray(features, np.float32)
    W = np.asarray(W, np.float32)
    bvec = np.asarray(b, np.float32).reshape(H)
    trans = np.asarray(transition, np.float32)
    masks_np = np.asarray(masks, np.float32)
    tags_np = np.asarray(tags).astype(np.int64)

    # prescale: typical per-step log-gain so exp-domain state stays in range
    tr64 = trans.astype(np.float64)
    finite = tr64 > NEG / 2
    row_lse = []
    for i in range(H):
        r = tr64[i][finite[i]]
        if r.size:
            m = r.max()
            row_lse.append(m + np.log(np.exp(r - m).sum()))
    c = float(np.mean(row_lse))

    est_host = np.ascontiguousarray(np.exp((trans - c).astype(np.float32)).T)
    wt_host = np.ascontiguousarray(W.T)
    q0_host = np.zeros((H, BL), np.float32)
    q0_host[START, :] = 1.0
    bcol_host = np.ascontiguousarray(bvec.reshape(H, 1))
    ones_h1 = np.ones((H, 1), np.float32)
    ones_1h = np.ones((1, H), np.float32)

    shared = dict(
        wt=wt_host, est=est_host, q0=q0_host, bcol=bcol_host,
        ones_h1=ones_h1, ones_1h=ones_1h,
    )
    in_maps = []
    for core in range(NCORES):
        fc = features[core * BL : (core + 1) * BL]          # [BL, T, F]
        ftr = np.ascontiguousarray(fc.transpose(2, 1, 0))    # [F, T, BL]
        in_maps.append(dict(shared, feats=ftr.reshape(F, T * BL)))

    nc = _get_program()
    res = run_bass_kernel_spmd(
        nc, in_maps, list(range(NCORES)),
        trace=bool(os.environ.get("CRF_TRACE")),
    )
    _CACHE["last_results"] = res

    # ---- host-side final assembly (O(B*T) index math only) ----
    stop_e = np.exp(tr64[STOP])                              # [H], 0 at START/STOP
    tags_ext = np.concatenate(
        [np.full((B, 1), START, np.int64), tags_np], axis=1
    )                                                        # [B, T+1]
    trans_sc = tr64[tags_ext[:, 1:], tags_ext[:, :-1]]       # [B, T]
    last_pos = masks_np.sum(axis=1).astype(np.int64)
    last_tag = np.take_along_axis(tags_ext, last_pos[:, None], axis=1)[:, 0]
    last_score = tr64[STOP, last_tag]                        # [B]

    fwd = np.zeros(B, np.float64)
    gold = np.zeros(B, np.float64)
    tidx = np.arange(T)
    for core in range(NCORES):
        out = res.results[core]
        em = np.asarray(out["emit"]).reshape(H, T, BL)
        qf = np.asarray(out["qfin"]).astype(np.float64)      # [H, BL]
        rh = np.asarray(out["rhist"]).reshape(NREN, BL).astype(np.float64)
        for bb in range(BL):
            g = core * BL + bb
            fwd[g] = (
                np.log(stop_e @ qf[:, bb])
                - np.log(rh[:, bb]).sum()
                + c * T
            )
            tg = tags_np[g]
            emit_sc = em[tg, tidx, bb].astype(np.float64) + bvec[tg]
            gold[g] = ((emit_sc + trans_sc[g]) * masks_np[g]).sum() + last_score[g]

    return np.float32(np.mean(fwd - gold))


# revision 7
# speedup vs baseline: 2.4630x; 2.4630x over previous
"""CRF NLL loss kernel for Trainium2 (8 NeuronCores, SPMD data-parallel over batch).

loss = mean_b(logZ_b - gold_b) for a linear-chain CRF, H=52 states, T=512,
B=64, F=1024.

Per core (8 sequences):
  - emit = features @ W.T on the PE in bf16 (fp32 PSUM accumulation), with the
    weight columns duplicated so emissions appear on partitions 0:52 AND
    64:116 (the backward half must be partition-aligned at 64).
  - logZ via a BIDIRECTIONAL forward algorithm in the exp domain, forward from
    START and backward from STOP simultaneously, meeting at T/2. Both
    recursions advance in one [128,128] block-diagonal bf16 matmul plus one
    [128,8] vector multiply per slot (the backward half reads emissions
    time-reversed — the host packs the second half of the feature columns in
    reverse time order so no negative strides are needed). 256 sequential
    slots instead of 512.
  - joint sum-renormalization every RENORM slots (the reciprocal is recorded
    and applied, so host bookkeeping is exact regardless of rounding).
  - raw emit goes back to DRAM; the gold-score gather (pure index math) and
    the final scalar assembly happen on host in float64.
"""

import os
import numpy as np

B, T, F, NT = 64, 512, 1024, 50
H = NT + 2
HB = 128                   # padded merged-state height
BO = 64                    # backward block partition offset
START, STOP = H - 2, H - 1
NEG = -100000000.0

NCORES = 8
BL = B // NCORES           # 8 sequences per core
HALF = T // 2              # 256 sequential slots
TCHUNK = 64                # slots per emit tile (TCHUNK * BL = 512 free)
NTILES = T // TCHUNK       # 8 emit tiles per core (4 fwd + 4 bwd)
KC = F // 128              # 8 contraction chunks
RENORM = 64                # joint renorm every RENORM slots
NREN = HALF // RENORM      # 4 renorms

_CACHE = {}


def _build_program():
    import concourse.bacc as bacc
    import concourse.tile as tile
    import concourse.mybir as mybir
    from concourse.bass import ts

    f32 = mybir.dt.float32
    bf16 = mybir.dt.bfloat16
    AF = mybir.ActivationFunctionType
    nc = bacc.Bacc("TRN2", target_bir_lowering=False, debug=False)

    feats = nc.dram_tensor("feats", [F, T * BL], bf16, kind="ExternalInput")
    wt = nc.dram_tensor("wt", [F, HB], bf16, kind="ExternalInput")
    blk = nc.dram_tensor("blk", [HB, HB], bf16, kind="ExternalInput")
    q0d = nc.dram_tensor("q0", [HB, BL], bf16, kind="ExternalInput")
    stopd = nc.dram_tensor("stope", [H, BL], f32, kind="ExternalInput")
    bcold = nc.dram_tensor("bcol", [HB, 1], f32, kind="ExternalInput")
    ones_k = nc.dram_tensor("ones_k", [HB, 1], bf16, kind="ExternalInput")
    ones_m = nc.dram_tensor("ones_m", [1, HB], bf16, kind="ExternalInput")

    emit_out = nc.dram_tensor("emit", [H, T * BL], f32, kind="ExternalOutput")
    qfin_out = nc.dram_tensor("qfin", [H, BL], bf16, kind="ExternalOutput")
    vfin_out = nc.dram_tensor("vfin", [H, BL], f32, kind="ExternalOutput")
    rhist_out = nc.dram_tensor("rhist", [1, NREN, BL], bf16, kind="ExternalOutput")

    feats_r = feats.ap().rearrange(
        "(kc p) (j n) -> j p kc n", kc=KC, n=TCHUNK * BL
    )

    with tile.TileContext(nc) as tc:
        with (
            tc.tile_pool(name="singles", bufs=1) as singles,
            tc.tile_pool(name="fpool", bufs=3) as fpool,
            tc.tile_pool(name="qpool", bufs=4) as qpool,
            tc.tile_pool(name="eps_ps", bufs=2, space="PSUM") as eps_ps,
            tc.tile_pool(name="q_ps", bufs=3, space="PSUM") as q_ps,
            tc.tile_pool(name="z_ps", bufs=1, space="PSUM") as z_ps,
            tc.tile_pool(name="bc_ps", bufs=1, space="PSUM") as bc_ps,
        ):
            wt_sb = singles.tile([128, KC, HB], bf16)
            nc.sync.dma_start(wt_sb[:], wt.ap().rearrange("(kc p) h -> p kc h", kc=KC))
            blk_sb = singles.tile([HB, HB], bf16)
            nc.sync.dma_start(blk_sb[:], blk.ap())
            q0_sb = singles.tile([HB, BL], bf16)
            nc.sync.dma_start(q0_sb[:], q0d.ap())
            stop_sb = singles.tile([HB, BL], f32)
            nc.sync.dma_start(stop_sb[BO : BO + H, :], stopd.ap())
            b_sb = singles.tile([HB, 1], f32)
            nc.sync.dma_start(b_sb[:], bcold.ap())
            ok_sb = singles.tile([HB, 1], bf16)
            nc.sync.dma_start(ok_sb[:], ones_k.ap())
            om_sb = singles.tile([1, HB], bf16)
            nc.sync.dma_start(om_sb[:], ones_m.ap())
            rhist_sb = singles.tile([1, NREN, BL], bf16)
            eemit_sb = singles.tile([HB, HALF, BL], f32)
            # rows outside the two emission blocks must be finite zeros
            nc.vector.memset(eemit_sb[:], 0.0)

            # ---- emit projection; tiles interleaved fwd/bwd so slot s only
            # needs tiles (s//TCHUNK) and 4+(s//TCHUNK) ----
            for j in (0, 4, 1, 5, 2, 6, 3, 7):
                ft = fpool.tile([128, KC, TCHUNK * BL], bf16)
                nc.sync.dma_start(ft[:], feats_r[j])
                eps = eps_ps.tile([HB, TCHUNK * BL], f32)
                for kc in range(KC):
                    nc.tensor.matmul(
                        eps[:],
                        wt_sb[:, kc, :],
                        ft[:, kc, :],
                        start=(kc == 0),
                        stop=(kc == KC - 1),
                    )
                if j < 4:
                    lo, sl = 0, ts(j, TCHUNK)
                else:
                    lo, sl = BO, ts(j - 4, TCHUNK)
                nc.scalar.activation(
                    eemit_sb[lo : lo + H, sl, :],
                    eps[lo : lo + H].rearrange("p (t b) -> p t b", b=BL),
                    AF.Exp,
                    bias=b_sb[lo : lo + H],
                )
                em_sb = fpool.tile([H, TCHUNK * BL], f32, tag="emit_stage")
                nc.scalar.copy(em_sb[:], eps[:H])
                nc.sync.dma_start(emit_out.ap()[:, ts(j, TCHUNK * BL)], em_sb[:])

            # ---- bidirectional scan, 256 merged slots ----
            state = q0_sb
            for s in range(HALF):
                ps = q_ps.tile([HB, BL], f32, tag="ps")
                nc.tensor.matmul(ps[:], blk_sb[:], state[:], start=True, stop=True)
                qn = qpool.tile([HB, BL], bf16)
                nc.vector.tensor_mul(qn[:], ps[:], eemit_sb[:, s, :])
                if s == 0:
                    # backward boundary: v_T = stopE comes from SBUF, not PSUM
                    nc.vector.tensor_mul(
                        qn[BO : BO + H],
                        stop_sb[BO : BO + H],
                        eemit_sb[BO : BO + H, 0, :],
                    )
                state = qn
                if (s + 1) % RENORM == 0:
                    k = (s + 1) // RENORM - 1
                    zs = z_ps.tile([1, BL], f32)
                    nc.tensor.matmul(zs[:], ok_sb[:], state[:], start=True, stop=True)
                    # bf16 out is fine: the exact stored value is both applied
                    # to the state and logged by the host
                    with nc.allow_low_precision(reason="renorm factor, consistent bookkeeping"):
                        nc.vector.reciprocal(rhist_sb[:, k, :], zs[:])
                    bc = bc_ps.tile([HB, BL], f32)
                    nc.tensor.matmul(
                        bc[:], om_sb[:], rhist_sb[:, k, :], start=True, stop=True
                    )
                    qr = qpool.tile([HB, BL], bf16)
                    nc.vector.tensor_mul(qr[:], bc[:], state[:])
                    state = qr

            # one extra backward matmul: v_{T/2} = E'^T w_{T/2}
            psf = q_ps.tile([HB, BL], f32, tag="ps")
            nc.tensor.matmul(psf[:], blk_sb[:], state[:], start=True, stop=True)
            vf_sb = singles.tile([HB, BL], f32)
            nc.scalar.copy(vf_sb[BO : BO + H], psf[BO : BO + H])

            nc.sync.dma_start(qfin_out.ap(), state[:H])
            nc.sync.dma_start(vfin_out.ap(), vf_sb[BO : BO + H])
            nc.sync.dma_start(rhist_out.ap(), rhist_sb[:])

    nc.compile()
    return nc


def _get_program():
    if "nc" not in _CACHE:
        _CACHE["nc"] = _build_program()
    return _CACHE["nc"]


def kernel(features, W, b, transition, masks, tags):
    import ml_dtypes
    from concourse.bass_utils import run_bass_kernel_spmd

    bf = ml_dtypes.bfloat16
    features = np.asarray(features, np.float32)
    W = np.asarray(W, np.float32)
    bvec = np.asarray(b, np.float32).reshape(H)
    trans = np.asarray(transition, np.float32)
    masks_np = np.asarray(masks, np.float32)
    tags_np = np.asarray(tags).astype(np.int64)

    # prescale: typical per-step log-gain keeps the exp-domain state in range
    tr64 = trans.astype(np.float64)
    finite = tr64 > NEG / 2
    row_lse = []
    for i in range(H):
        r = tr64[i][finite[i]]
        if r.size:
            m = r.max()
            row_lse.append(m + np.log(np.exp(r - m).sum()))
    c = float(np.mean(row_lse))

    Ef = np.exp((trans - c).astype(np.float32)).astype(bf)   # [i,j]
    blk_host = np.zeros((HB, HB), bf)
    blk_host[:H, :H] = Ef.T                                  # fwd: E' q
    blk_host[BO : BO + H, BO : BO + H] = Ef                  # bwd: E'^T w
    wt_host = np.zeros((F, HB), bf)
    wt_host[:, :H] = W.T.astype(bf)
    wt_host[:, BO : BO + H] = wt_host[:, :H]
    q0_host = np.zeros((HB, BL), bf)
    q0_host[START, :] = 1.0
    stop_host = np.broadcast_to(
        np.exp(tr64[STOP]).astype(np.float32)[:, None], (H, BL)
    ).copy()
    bcol_host = np.zeros((HB, 1), np.float32)
    bcol_host[:H, 0] = bvec
    bcol_host[BO : BO + H, 0] = bvec
    ones_k = np.ones((HB, 1), bf)
    ones_m = np.ones((1, HB), bf)

    shared = dict(
        wt=wt_host, blk=blk_host, q0=q0_host, stope=stop_host,
        bcol=bcol_host, ones_k=ones_k, ones_m=ones_m,
    )
    in_maps = []
    for core in range(NCORES):
        fc = features[core * BL : (core + 1) * BL]           # [BL, T, F]
        ftr = fc.transpose(2, 1, 0)                          # [F, T, BL]
        fwd_half = ftr[:, :HALF, :]                          # t ascending
        bwd_half = ftr[:, : HALF - 1 : -1, :]                # t = T-1 .. T/2
        packed = np.concatenate([fwd_half, bwd_half], axis=1)
        in_maps.append(
            dict(shared, feats=np.ascontiguousarray(packed).reshape(F, T * BL).astype(bf))
        )

    nc = _get_program()
    res = run_bass_kernel_spmd(
        nc, in_maps, list(range(NCORES)),
        trace=bool(os.environ.get("CRF_TRACE")),
    )
    _CACHE["last_results"] = res

    # ---- host-side final assembly ----
    tags_ext = np.concatenate(
        [np.full((B, 1), START, np.int64), tags_np], axis=1
    )
    trans_sc = tr64[tags_ext[:, 1:], tags_ext[:, :-1]]       # [B, T]
    last_pos = masks_np.sum(axis=1).astype(np.int64)
    last_tag = np.take_along_axis(tags_ext, last_pos[:, None], axis=1)[:, 0]
    last_score = tr64[STOP, last_tag]

    # emit column base for each t: fwd half packed first, then reversed bwd half
    tcols = np.where(
        np.arange(T) < HALF, np.arange(T) * BL, (T - 1 - np.arange(T) + HALF) * BL
    )

    fwd = np.zeros(B, np.float64)
    gold = np.zeros(B, np.float64)
    for core in range(NCORES):
        out = res.results[core]
        em = np.asarray(out["emit"])                         # [H, T*BL] f32
        qf = np.asarray(out["qfin"]).astype(np.float64)      # [H, BL]
        vf = np.asarray(out["vfin"]).astype(np.float64)      # [H, BL]
        rh = np.asarray(out["rhist"]).reshape(NREN, BL).astype(np.float64)
        for bb in range(BL):
            g = core * BL + bb
            fwd[g] = (
                np.log((qf[:, bb] * vf[:, bb]).sum())
                - 2.0 * np.log(rh[:, bb]).sum()
                + c * T
            )
            tg = tags_np[g]
            emit_sc = em[tg, tcols + bb].astype(np.float64) + bvec[tg]
            gold[g] = ((emit_sc + trans_sc[g]) * masks_np[g]).sum() + last_score[g]

    return np.float32(np.mean(fwd - gold))


# revision 15
# speedup vs baseline: 2.4691x; 1.0025x over previous
"""CRF NLL loss kernel for Trainium2 (8 NeuronCores, SPMD data-parallel over batch).

loss = mean_b(logZ_b - gold_b) for a linear-chain CRF, H=52 states, T=512,
B=64, F=1024.

Per core (8 sequences):
  - emit = features @ W.T on the PE in bf16 (fp32 PSUM accumulation), with the
    weight columns duplicated so emissions appear on partitions 0:52 AND
    64:116 (the backward half must be partition-aligned at 64).
  - logZ via a BIDIRECTIONAL forward algorithm in the exp domain, forward from
    START and backward from STOP simultaneously, meeting at T/2. Both
    recursions advance in one [128,128] block-diagonal bf16 matmul plus one
    [128,8] vector multiply per slot (the backward half reads emissions
    time-reversed — the host packs the second half of the feature columns in
    reverse time order so no negative strides are needed). 256 sequential
    slots instead of 512.
  - joint sum-renormalization every RENORM slots (the reciprocal is recorded
    and applied, so host bookkeeping is exact regardless of rounding).
  - raw emit goes back to DRAM; the gold-score gather (pure index math) and
    the final scalar assembly happen on host in float64.
"""

import os
import numpy as np

B, T, F, NT = 64, 512, 1024, 50
H = NT + 2
HB = 128                   # padded merged-state height
BO = 64                    # backward block partition offset
START, STOP = H - 2, H - 1
NEG = -100000000.0

NCORES = 8
BL = B // NCORES           # 8 sequences per core
HALF = T // 2              # 256 sequential slots
TCHUNK = 64                # slots per emit tile (TCHUNK * BL = 512 free)
NTILES = T // TCHUNK       # 8 emit tiles per core (4 fwd + 4 bwd)
KC = F // 128              # 8 contraction chunks
RENORM = 128               # joint renorm every RENORM slots
NREN = HALF // RENORM      # 2 renorms

_CACHE = {}


def _build_program():
    import concourse.bacc as bacc
    import concourse.tile as tile
    import concourse.mybir as mybir
    from concourse.bass import ts

    f32 = mybir.dt.float32
    bf16 = mybir.dt.bfloat16
    AF = mybir.ActivationFunctionType
    nc = bacc.Bacc("TRN2", target_bir_lowering=False, debug=False)

    feats = nc.dram_tensor(
        "feats", [NTILES, 128, KC, TCHUNK * BL], bf16, kind="ExternalInput"
    )
    wt = nc.dram_tensor("wt", [F, HB], bf16, kind="ExternalInput")
    blk = nc.dram_tensor("blk", [HB, HB], bf16, kind="ExternalInput")
    q0d = nc.dram_tensor("q0", [HB, BL], bf16, kind="ExternalInput")
    stopd = nc.dram_tensor("stope", [H, BL], f32, kind="ExternalInput")
    bcold = nc.dram_tensor("bcol", [HB, 1], f32, kind="ExternalInput")
    ones_k = nc.dram_tensor("ones_k", [HB, 1], bf16, kind="ExternalInput")
    ones_m = nc.dram_tensor("ones_m", [1, HB], bf16, kind="ExternalInput")

    emit_out = nc.dram_tensor("emit", [H, T * BL], f32, kind="ExternalOutput")
    qfin_out = nc.dram_tensor("qfin", [H, BL], bf16, kind="ExternalOutput")
    vfin_out = nc.dram_tensor("vfin", [H, BL], f32, kind="ExternalOutput")
    rhist_out = nc.dram_tensor("rhist", [1, NREN, BL], bf16, kind="ExternalOutput")

    feats_r = feats.ap()

    with tile.TileContext(nc) as tc:
        with (
            tc.tile_pool(name="singles", bufs=1) as singles,
            tc.tile_pool(name="fpool", bufs=NTILES) as fpool,
            tc.tile_pool(name="empool", bufs=2) as empool,
            tc.tile_pool(name="qpool", bufs=4) as qpool,
            tc.tile_pool(name="eps_ps", bufs=2, space="PSUM") as eps_ps,
            tc.tile_pool(name="q_ps", bufs=3, space="PSUM") as q_ps,
            tc.tile_pool(name="z_ps", bufs=1, space="PSUM") as z_ps,
            tc.tile_pool(name="bc_ps", bufs=1, space="PSUM") as bc_ps,
        ):
            wt_sb = singles.tile([128, KC, HB], bf16)
            nc.sync.dma_start(wt_sb[:], wt.ap().rearrange("(kc p) h -> p kc h", kc=KC))
            blk_sb = singles.tile([HB, HB], bf16)
            nc.sync.dma_start(blk_sb[:], blk.ap())
            q0_sb = singles.tile([HB, BL], bf16)
            nc.sync.dma_start(q0_sb[:], q0d.ap())
            stop_sb = singles.tile([HB, BL], f32)
            nc.sync.dma_start(stop_sb[BO : BO + H, :], stopd.ap())
            b_sb = singles.tile([HB, 1], f32)
            nc.sync.dma_start(b_sb[:], bcold.ap())
            ok_sb = singles.tile([HB, 1], bf16)
            nc.sync.dma_start(ok_sb[:], ones_k.ap())
            om_sb = singles.tile([1, HB], bf16)
            nc.sync.dma_start(om_sb[:], ones_m.ap())
            rhist_sb = singles.tile([1, NREN, BL], bf16)
            eemit_sb = singles.tile([HB, HALF, BL], f32)
            # rows outside the two emission blocks must be finite zeros
            nc.vector.memset(eemit_sb[:], 0.0)
            # preload the exp spline table while the feature DMAs run
            warm_sb = singles.tile([1, 1], f32)
            nc.scalar.activation(warm_sb[:], b_sb[:1, :], AF.Exp)

            # ---- emit projection; tiles interleaved fwd/bwd so slot s only
            # needs tiles (s//TCHUNK) and 4+(s//TCHUNK). All feature DMAs are
            # issued up front, alternating between the two HWDGE rings. ----
            order = (0, 4, 1, 5, 2, 6, 3, 7)
            fts = {}
            for idx, j in enumerate(order):
                fts[j] = fpool.tile(
                    [128, KC, TCHUNK * BL], bf16, name=f"ft{j}", tag="ft"
                )
                eng = nc.sync if idx % 2 == 0 else nc.scalar
                eng.dma_start(fts[j][:], feats_r[j])
            for idx, j in enumerate(order):
                ft = fts[j]
                eps = eps_ps.tile([HB, TCHUNK * BL], f32)
                # head tiles (0,4) need full-width matmuls for minimum latency;
                # later tiles use N=128 column groups that fit the ~250ns PE
                # gaps between scan steps without delaying them
                nsplit = 2 if idx < 2 else 4
                ncols = (TCHUNK * BL) // nsplit
                for ng in range(nsplit):
                    cs = slice(ng * ncols, (ng + 1) * ncols)
                    for kc in range(KC):
                        nc.tensor.matmul(
                            eps[:, cs],
                            wt_sb[:, kc, :],
                            ft[:, kc, cs],
                            start=(kc == 0),
                            stop=(kc == KC - 1),
                        )
                if j < 4:
                    lo, sl = 0, ts(j, TCHUNK)
                else:
                    lo, sl = BO, ts(j - 4, TCHUNK)
                nc.scalar.activation(
                    eemit_sb[lo : lo + H, sl, :],
                    eps[lo : lo + H].rearrange("p (t b) -> p t b", b=BL),
                    AF.Exp,
                    bias=b_sb[lo : lo + H],
                )
                em_sb = empool.tile([H, TCHUNK * BL], f32, tag="emit_stage")
                nc.scalar.copy(em_sb[:], eps[:H])
                nc.scalar.dma_start(emit_out.ap()[:, ts(j, TCHUNK * BL)], em_sb[:])

            # ---- bidirectional scan, 256 merged slots ----
            state = q0_sb
            for s in range(HALF):
                ps = q_ps.tile([HB, BL], f32, tag="ps")
                nc.tensor.matmul(ps[:], blk_sb[:], state[:], start=True, stop=True)
                qn = qpool.tile([HB, BL], bf16)
                nc.vector.tensor_mul(qn[:], ps[:], eemit_sb[:, s, :])
                if s == 0:
                    # backward boundary: v_T = stopE comes from SBUF, not PSUM
                    nc.vector.tensor_mul(
                        qn[BO : BO + H],
                        stop_sb[BO : BO + H],
                        eemit_sb[BO : BO + H, 0, :],
                    )
                state = qn
                if (s + 1) % RENORM == 0:
                    k = (s + 1) // RENORM - 1
                    zs = z_ps.tile([1, BL], f32)
                    nc.tensor.matmul(zs[:], ok_sb[:], state[:], start=True, stop=True)
                    # bf16 out is fine: the exact stored value is both applied
                    # to the state and logged by the host
                    with nc.allow_low_precision(reason="renorm factor, consistent bookkeeping"):
                        nc.vector.reciprocal(rhist_sb[:, k, :], zs[:])
                    bc = bc_ps.tile([HB, BL], f32)
                    nc.tensor.matmul(
                        bc[:], om_sb[:], rhist_sb[:, k, :], start=True, stop=True
                    )
                    qr = qpool.tile([HB, BL], bf16)
                    nc.vector.tensor_mul(qr[:], bc[:], state[:])
                    state = qr

            # one extra backward matmul: v_{T/2} = E'^T w_{T/2}
            psf = q_ps.tile([HB, BL], f32, tag="ps")
            nc.tensor.matmul(psf[:], blk_sb[:], state[:], start=True, stop=True)
            vf_sb = singles.tile([HB, BL], f32)
            nc.scalar.copy(vf_sb[BO : BO + H], psf[BO : BO + H])

            nc.sync.dma_start(qfin_out.ap(), state[:H])
            nc.sync.dma_start(vfin_out.ap(), vf_sb[BO : BO + H])
            nc.sync.dma_start(rhist_out.ap(), rhist_sb[:])

    nc.compile()
    return nc


def _get_program():
    if "nc" not in _CACHE:
        _CACHE["nc"] = _build_program()
    return _CACHE["nc"]


def kernel(features, W, b, transition, masks, tags):
    import ml_dtypes
    from concourse.bass_utils import run_bass_kernel_spmd

    bf = ml_dtypes.bfloat16
    features = np.asarray(features, np.float32)
    W = np.asarray(W, np.float32)
    bvec = np.asarray(b, np.float32).reshape(H)
    trans = np.asarray(transition, np.float32)
    masks_np = np.asarray(masks, np.float32)
    tags_np = np.asarray(tags).astype(np.int64)

    # prescale: typical per-step log-gain keeps the exp-domain state in range
    tr64 = trans.astype(np.float64)
    finite = tr64 > NEG / 2
    row_lse = []
    for i in range(H):
        r = tr64[i][finite[i]]
        if r.size:
            m = r.max()
            row_lse.append(m + np.log(np.exp(r - m).sum()))
    c = float(np.mean(row_lse))

    Ef = np.exp((trans - c).astype(np.float32)).astype(bf)   # [i,j]
    blk_host = np.zeros((HB, HB), bf)
    blk_host[:H, :H] = Ef.T                                  # fwd: E' q
    blk_host[BO : BO + H, BO : BO + H] = Ef                  # bwd: E'^T w
    wt_host = np.zeros((F, HB), bf)
    wt_host[:, :H] = W.T.astype(bf)
    wt_host[:, BO : BO + H] = wt_host[:, :H]
    q0_host = np.zeros((HB, BL), bf)
    q0_host[START, :] = 1.0
    stop_host = np.broadcast_to(
        np.exp(tr64[STOP]).astype(np.float32)[:, None], (H, BL)
    ).copy()
    bcol_host = np.zeros((HB, 1), np.float32)
    bcol_host[:H, 0] = bvec
    bcol_host[BO : BO + H, 0] = bvec
    ones_k = np.ones((HB, 1), bf)
    ones_m = np.ones((1, HB), bf)

    shared = dict(
        wt=wt_host, blk=blk_host, q0=q0_host, stope=stop_host,
        bcol=bcol_host, ones_k=ones_k, ones_m=ones_m,
    )
    in_maps = []
    for core in range(NCORES):
        fc = features[core * BL : (core + 1) * BL]           # [BL, T, F]
        ftr = fc.transpose(2, 1, 0)                          # [F, T, BL]
        fwd_half = ftr[:, :HALF, :]                          # t ascending
        bwd_half = ftr[:, : HALF - 1 : -1, :]                # t = T-1 .. T/2
        packed = np.concatenate([fwd_half, bwd_half], axis=1)  # [F, T, BL]
        # device layout: [tile_j, partition, kc, tchunk*bl], each tile a
        # contiguous 512KB block (8KB contiguous per partition → efficient DMA)
        pk = packed.reshape(KC, 128, NTILES, TCHUNK * BL).transpose(2, 1, 0, 3)
        in_maps.append(
            dict(shared, feats=np.ascontiguousarray(pk).astype(bf))
        )

    nc = _get_program()
    res = run_bass_kernel_spmd(
        nc, in_maps, list(range(NCORES)),
        trace=bool(os.environ.get("CRF_TRACE")),
    )
    _CACHE["last_results"] = res

    # ---- host-side final assembly ----
    tags_ext = np.concatenate(
        [np.full((B, 1), START, np.int64), tags_np], axis=1
    )
    trans_sc = tr64[tags_ext[:, 1:], tags_ext[:, :-1]]       # [B, T]
    last_pos = masks_np.sum(axis=1).astype(np.int64)
    last_tag = np.take_along_axis(tags_ext, last_pos[:, None], axis=1)[:, 0]
    last_score = tr64[STOP, last_tag]

    # emit column base for each t: fwd half packed first, then reversed bwd half
    tcols = np.where(
        np.arange(T) < HALF, np.arange(T) * BL, (T - 1 - np.arange(T) + HALF) * BL
    )

    fwd = np.zeros(B, np.float64)
    gold = np.zeros(B, np.float64)
    for core in range(NCORES):
        out = res.results[core]
        em = np.asarray(out["emit"])                         # [H, T*BL] f32
        qf = np.asarray(out["qfin"]).astype(np.float64)      # [H, BL]
        vf = np.asarray(out["vfin"]).astype(np.float64)      # [H, BL]
        rh = np.asarray(out["rhist"]).reshape(NREN, BL).astype(np.float64)
        for bb in range(BL):
            g = core * BL + bb
            fwd[g] = (
                np.log((qf[:, bb] * vf[:, bb]).sum())
                - 2.0 * np.log(rh[:, bb]).sum()
                + c * T
            )
            tg = tags_np[g]
            emit_sc = em[tg, tcols + bb].astype(np.float64) + bvec[tg]
            gold[g] = ((emit_sc + trans_sc[g]) * masks_np[g]).sum() + last_score[g]

    return np.float32(np.mean(fwd - gold))


# revision 17
# speedup vs baseline: 2.6756x; 1.0836x over previous
"""CRF NLL loss kernel for Trainium2 (8 NeuronCores, SPMD data-parallel over batch).

loss = mean_b(logZ_b - gold_b) for a linear-chain CRF, H=52 states, T=512,
B=64, F=1024.

Per core (8 sequences):
  - emit = features @ W.T on the PE in bf16 (fp32 PSUM accumulation), with the
    weight columns duplicated so emissions appear on partitions 0:52 AND
    64:116 (the backward half must be partition-aligned at 64).
  - logZ via a BIDIRECTIONAL forward algorithm in the exp domain, forward from
    START and backward from STOP simultaneously, meeting at T/2. Both
    recursions advance in one [128,128] block-diagonal bf16 matmul plus one
    [128,8] vector multiply per slot (the backward half reads emissions
    time-reversed — the host packs the second half of the feature columns in
    reverse time order so no negative strides are needed). 256 sequential
    slots instead of 512.
  - joint sum-renormalization every RENORM slots (the reciprocal is recorded
    and applied, so host bookkeeping is exact regardless of rounding).
  - raw emit goes back to DRAM; the gold-score gather (pure index math) and
    the final scalar assembly happen on host in float64.
"""

import os
import numpy as np

B, T, F, NT = 64, 512, 1024, 50
H = NT + 2
HB = 128                   # padded merged-state height
BO = 64                    # backward block partition offset
START, STOP = H - 2, H - 1
NEG = -100000000.0

NCORES = 8
BL = B // NCORES           # 8 sequences per core
HALF = T // 2              # 256 sequential slots
TCHUNK = 64                # slots per emit tile (TCHUNK * BL = 512 free)
NTILES = T // TCHUNK       # 8 emit tiles per core (4 fwd + 4 bwd)
KC = F // 128              # 8 contraction chunks
RENORM = 128               # joint renorm every RENORM slots
NREN = HALF // RENORM      # 2 renorms

_CACHE = {}


def _build_program():
    import concourse.bacc as bacc
    import concourse.tile as tile
    import concourse.mybir as mybir
    from concourse.bass import ts

    f32 = mybir.dt.float32
    bf16 = mybir.dt.bfloat16
    AF = mybir.ActivationFunctionType
    nc = bacc.Bacc("TRN2", target_bir_lowering=False, debug=False)

    feats = nc.dram_tensor(
        "feats", [NTILES, 128, KC, TCHUNK * BL], bf16, kind="ExternalInput"
    )
    wt = nc.dram_tensor("wt", [F, HB], bf16, kind="ExternalInput")
    blk = nc.dram_tensor("blk", [HB, HB], bf16, kind="ExternalInput")
    q0d = nc.dram_tensor("q0", [HB, BL], bf16, kind="ExternalInput")
    stopd = nc.dram_tensor("stope", [H, BL], f32, kind="ExternalInput")
    bcold = nc.dram_tensor("bcol", [HB, 1], f32, kind="ExternalInput")
    ones_k = nc.dram_tensor("ones_k", [HB, 1], bf16, kind="ExternalInput")
    ones_m = nc.dram_tensor("ones_m", [1, HB], bf16, kind="ExternalInput")

    emit_out = nc.dram_tensor("emit", [H, T * BL], f32, kind="ExternalOutput")
    qfin_out = nc.dram_tensor("qfin", [H, BL], bf16, kind="ExternalOutput")
    vfin_out = nc.dram_tensor("vfin", [H, BL], f32, kind="ExternalOutput")
    rhist_out = nc.dram_tensor("rhist", [1, NREN, BL], bf16, kind="ExternalOutput")

    feats_r = feats.ap()

    with tile.TileContext(nc) as tc:
        with (
            tc.tile_pool(name="singles", bufs=1) as singles,
            tc.tile_pool(name="fpool", bufs=NTILES) as fpool,
            tc.tile_pool(name="empool", bufs=2) as empool,
            tc.tile_pool(name="qpool", bufs=4) as qpool,
            tc.tile_pool(name="eps_ps", bufs=2, space="PSUM") as eps_ps,
            tc.tile_pool(name="q_ps", bufs=2, space="PSUM") as q_ps,
            tc.tile_pool(name="z_ps", bufs=1, space="PSUM") as z_ps,
            tc.tile_pool(name="bc_ps", bufs=1, space="PSUM") as bc_ps,
        ):
            # feature-tile DMAs are the longest pole: issue the head tiles
            # (0, 4) first, one per HWDGE ring, params immediately behind
            fts = {}
            for j in (0, 4, 1, 5, 2, 6, 3, 7):
                fts[j] = fpool.tile(
                    [128, KC, TCHUNK * BL], bf16, name=f"ft{j}", tag="ft"
                )
            wt_sb = singles.tile([128, KC, HB], bf16)
            blk_sb = singles.tile([HB, HB], bf16)
            q0_sb = singles.tile([HB, BL], bf16)
            stop_sb = singles.tile([HB, BL], f32)
            b_sb = singles.tile([HB, 1], f32)
            ok_sb = singles.tile([HB, 1], bf16)
            om_sb = singles.tile([1, HB], bf16)

            nc.sync.dma_start(fts[0][:], feats_r[0])
            nc.scalar.dma_start(wt_sb[:], wt.ap().rearrange("(kc p) h -> p kc h", kc=KC))
            nc.scalar.dma_start(fts[4][:], feats_r[4])
            nc.sync.dma_start(blk_sb[:], blk.ap())
            nc.sync.dma_start(q0_sb[:], q0d.ap())
            nc.sync.dma_start(stop_sb[BO : BO + H, :], stopd.ap())
            nc.sync.dma_start(b_sb[:], bcold.ap())
            nc.sync.dma_start(ok_sb[:], ones_k.ap())
            nc.sync.dma_start(om_sb[:], ones_m.ap())
            for j, eng in ((1, nc.sync), (5, nc.scalar), (2, nc.sync),
                           (6, nc.scalar), (3, nc.sync), (7, nc.scalar)):
                eng.dma_start(fts[j][:], feats_r[j])

            rhist_sb = singles.tile([1, NREN, BL], bf16)
            eemit_sb = singles.tile([HB, HALF, BL], f32)
            # rows outside the two emission blocks must be finite zeros
            nc.vector.memset(eemit_sb[:], 0.0)
            # preload the exp spline table while the feature DMAs run
            warm_sb = singles.tile([1, 2], f32)
            nc.vector.memset(warm_sb[:, :1], 0.0)
            nc.scalar.activation(warm_sb[:, 1:], warm_sb[:, :1], AF.Exp)

            def emit_tile_mms(j, nsplit, ng):
                """column-group ng of tile j's emit matmuls (8 accumulating)"""
                ncols = (TCHUNK * BL) // nsplit
                cs = slice(ng * ncols, (ng + 1) * ncols)
                for kc in range(KC):
                    nc.tensor.matmul(
                        eps_tiles[j][:, cs],
                        wt_sb[:, kc, :],
                        fts[j][:, kc, cs],
                        start=(kc == 0),
                        stop=(kc == KC - 1),
                    )

            def emit_tile_finish(j, nsplit, ng):
                """exp column-group ng into the eemit buffer"""
                ncols = TCHUNK // nsplit
                if j < 4:
                    lo, s0 = 0, j * TCHUNK + ng * ncols
                else:
                    lo, s0 = BO, (j - 4) * TCHUNK + ng * ncols
                eps3 = eps_tiles[j].rearrange("p (t b) -> p t b", b=BL)
                nc.scalar.activation(
                    eemit_sb[lo : lo + H, s0 : s0 + ncols, :],
                    eps3[lo : lo + H, ng * ncols : (ng + 1) * ncols, :],
                    AF.Exp,
                    bias=b_sb[lo : lo + H],
                )

            def emit_tile_out(j):
                """raw emit back to DRAM for the host-side gold gather"""
                em_sb = empool.tile(
                    [H, TCHUNK * BL], f32, name=f"em{j}", tag="emit_stage"
                )
                nc.scalar.copy(em_sb[:], eps_tiles[j][:H])
                nc.scalar.dma_start(emit_out.ap()[:, ts(j, TCHUNK * BL)], em_sb[:])

            eps_tiles = {}
            for j in (0, 4, 1, 5, 2, 6, 3, 7):
                eps_tiles[j] = eps_ps.tile(
                    [HB, TCHUNK * BL], f32, name=f"eps{j}", tag=f"eps{j % 2}"
                )

            # head tiles 0 and 4, first halves first so slot 0 starts early
            for j, ng in ((0, 0), (4, 0), (0, 1), (4, 1)):
                emit_tile_mms(j, 2, ng)
                emit_tile_finish(j, 2, ng)
            emit_tile_out(0)
            emit_tile_out(4)

            # remaining tiles: one N=128 matmul group (~200ns) per scan slot,
            # paced to finish just before the scan needs each tile
            pending = []
            for j in (1, 5, 2, 6, 3, 7):
                for ng in range(4):
                    pending.append((emit_tile_mms, (j, 4, ng)))
                    pending.append((emit_tile_finish, (j, 4, ng)))
                pending.append((emit_tile_out, (j,)))
            pending.reverse()  # pop from the end

            # ---- bidirectional scan, 256 merged slots ----
            state = q0_sb
            for s in range(HALF):
                ps = q_ps.tile([HB, BL], f32, tag="ps")
                nc.tensor.matmul(ps[:], blk_sb[:], state[:], start=True, stop=True)
                qn = qpool.tile([HB, BL], bf16)
                nc.vector.tensor_mul(qn[:], ps[:], eemit_sb[:, s, :])
                if s == 0:
                    # backward boundary: v_T = stopE comes from SBUF, not PSUM
                    nc.vector.tensor_mul(
                        qn[BO : BO + H],
                        stop_sb[BO : BO + H],
                        eemit_sb[BO : BO + H, 0, :],
                    )
                state = qn
                npop = 2 if s % 8 == 0 else 1
                for _ in range(npop):
                    if pending:
                        fn, args = pending.pop()
                        fn(*args)
                if (s + 1) % RENORM == 0:
                    k = (s + 1) // RENORM - 1
                    zs = z_ps.tile([1, BL], f32)
                    nc.tensor.matmul(zs[:], ok_sb[:], state[:], start=True, stop=True)
                    # bf16 out is fine: the exact stored value is both applied
                    # to the state and logged by the host
                    with nc.allow_low_precision(reason="renorm factor, consistent bookkeeping"):
                        nc.vector.reciprocal(rhist_sb[:, k, :], zs[:])
                    bc = bc_ps.tile([HB, BL], f32)
                    nc.tensor.matmul(
                        bc[:], om_sb[:], rhist_sb[:, k, :], start=True, stop=True
                    )
                    qr = qpool.tile([HB, BL], bf16)
                    nc.vector.tensor_mul(qr[:], bc[:], state[:])
                    state = qr

            # one extra backward matmul: v_{T/2} = E'^T w_{T/2}
            psf = q_ps.tile([HB, BL], f32, tag="ps")
            nc.tensor.matmul(psf[:], blk_sb[:], state[:], start=True, stop=True)
            vf_sb = singles.tile([HB, BL], f32)
            nc.scalar.copy(vf_sb[BO : BO + H], psf[BO : BO + H])

            nc.sync.dma_start(qfin_out.ap(), state[:H])
            nc.sync.dma_start(vfin_out.ap(), vf_sb[BO : BO + H])
            nc.sync.dma_start(rhist_out.ap(), rhist_sb[:])

    nc.compile()
    return nc


def _get_program():
    if "nc" not in _CACHE:
        _CACHE["nc"] = _build_program()
    return _CACHE["nc"]


def kernel(features, W, b, transition, masks, tags):
    import ml_dtypes
    from concourse.bass_utils import run_bass_kernel_spmd

    bf = ml_dtypes.bfloat16
    features = np.asarray(features, np.float32)
    W = np.asarray(W, np.float32)
    bvec = np.asarray(b, np.float32).reshape(H)
    trans = np.asarray(transition, np.float32)
    masks_np = np.asarray(masks, np.float32)
    tags_np = np.asarray(tags).astype(np.int64)

    # prescale: typical per-step log-gain keeps the exp-domain state in range
    tr64 = trans.astype(np.float64)
    finite = tr64 > NEG / 2
    row_lse = []
    for i in range(H):
        r = tr64[i][finite[i]]
        if r.size:
            m = r.max()
            row_lse.append(m + np.log(np.exp(r - m).sum()))
    c = float(np.mean(row_lse))

    Ef = np.exp((trans - c).astype(np.float32)).astype(bf)   # [i,j]
    blk_host = np.zeros((HB, HB), bf)
    blk_host[:H, :H] = Ef.T                                  # fwd: E' q
    blk_host[BO : BO + H, BO : BO + H] = Ef                  # bwd: E'^T w
    wt_host = np.zeros((F, HB), bf)
    wt_host[:, :H] = W.T.astype(bf)
    wt_host[:, BO : BO + H] = wt_host[:, :H]
    q0_host = np.zeros((HB, BL), bf)
    q0_host[START, :] = 1.0
    stop_host = np.broadcast_to(
        np.exp(tr64[STOP]).astype(np.float32)[:, None], (H, BL)
    ).copy()
    bcol_host = np.zeros((HB, 1), np.float32)
    bcol_host[:H, 0] = bvec
    bcol_host[BO : BO + H, 0] = bvec
    ones_k = np.ones((HB, 1), bf)
    ones_m = np.ones((1, HB), bf)

    shared = dict(
        wt=wt_host, blk=blk_host, q0=q0_host, stope=stop_host,
        bcol=bcol_host, ones_k=ones_k, ones_m=ones_m,
    )
    in_maps = []
    for core in range(NCORES):
        fc = features[core * BL : (core + 1) * BL]           # [BL, T, F]
        ftr = fc.transpose(2, 1, 0)                          # [F, T, BL]
        fwd_half = ftr[:, :HALF, :]                          # t ascending
        bwd_half = ftr[:, : HALF - 1 : -1, :]                # t = T-1 .. T/2
        packed = np.concatenate([fwd_half, bwd_half], axis=1)  # [F, T, BL]
        # device layout: [tile_j, partition, kc, tchunk*bl], each tile a
        # contiguous 512KB block (8KB contiguous per partition → efficient DMA)
        pk = packed.reshape(KC, 128, NTILES, TCHUNK * BL).transpose(2, 1, 0, 3)
        in_maps.append(
            dict(shared, feats=np.ascontiguousarray(pk).astype(bf))
        )

    nc = _get_program()
    res = run_bass_kernel_spmd(
        nc, in_maps, list(range(NCORES)),
        trace=bool(os.environ.get("CRF_TRACE")),
    )
    _CACHE["last_results"] = res

    # ---- host-side final assembly ----
    tags_ext = np.concatenate(
        [np.full((B, 1), START, np.int64), tags_np], axis=1
    )
    trans_sc = tr64[tags_ext[:, 1:], tags_ext[:, :-1]]       # [B, T]
    last_pos = masks_np.sum(axis=1).astype(np.int64)
    last_tag = np.take_along_axis(tags_ext, last_pos[:, None], axis=1)[:, 0]
    last_score = tr64[STOP, last_tag]

    # emit column base for each t: fwd half packed first, then reversed bwd half
    tcols = np.where(
        np.arange(T) < HALF, np.arange(T) * BL, (T - 1 - np.arange(T) + HALF) * BL
    )

    fwd = np.zeros(B, np.float64)
    gold = np.zeros(B, np.float64)
    for core in range(NCORES):
        out = res.results[core]
        em = np.asarray(out["emit"])                         # [H, T*BL] f32
        qf = np.asarray(out["qfin"]).astype(np.float64)      # [H, BL]
        vf = np.asarray(out["vfin"]).astype(np.float64)      # [H, BL]
        rh = np.asarray(out["rhist"]).reshape(NREN, BL).astype(np.float64)
        for bb in range(BL):
            g = core * BL + bb
            fwd[g] = (
                np.log((qf[:, bb] * vf[:, bb]).sum())
                - 2.0 * np.log(rh[:, bb]).sum()
                + c * T
            )
            tg = tags_np[g]
            emit_sc = em[tg, tcols + bb].astype(np.float64) + bvec[tg]
            gold[g] = ((emit_sc + trans_sc[g]) * masks_np[g]).sum() + last_score[g]

    return np.float32(np.mean(fwd - gold))
